# revision 3
# baseline (speedup 1.0000x reference)
"""Trainium2 Bass kernel for DigitConvolutionalModel.

Reference computation (B = 32768):
    x: [B, 784] -> reshape [B, 28, 28]
    conv 3x3 valid with w_conv -> [B, 26, 26] -> [B, 676]
    h1 = relu(conv @ W1 + b1)    W1: [676, 100]
    h2 = relu(h1 @ W2 + b2)      W2: [100, 100]
    out = h2 @ W3 + b3           W3: [100, 10]

Strategy
--------
Pure data parallel: batch split 8 ways (4096 rows/core), weights replicated.
The conv is linear, so it is folded into W1 on the host:
    conv(x) @ W1 == x @ (M @ W1) = x @ W1e,  W1e: [784, 100]
removing the conv from the device entirely (exact up to fp rounding).

On-device layout is "transposed": features on SBUF partitions, batch on the
free dimension, so each layer's PSUM output feeds the next matmul directly
as the moving operand. x is cast to fp16 on the host (error stays ~1e-3 vs
the fp32 reference) and shipped as [128, 6, B_LOC] plus a resident 16-row
tail, so every DMA descriptor is a >=512B contiguous run.

v2 schedule (vs the 42.1us baseline):
  * x is fully RESIDENT in SBUF (6.3MB): one big DMA per batch group, all
    issued eagerly at t=0 across the sync+scalar rings. No buffer-slot
    throttling, so the 16 DMA queues stay ~100% busy instead of 58%.
  * PE clock warm-up: the TRN2 HAM clock gate runs the PE at 1.2GHz until
    it has been busy ~3.4us. Dummy matmuls on a memset tile (zero deps)
    burn that window during the initial DMA wait so every real matmul
    runs at 2.4GHz.
  * The scalar/ACT engine is not used at all (biases+relu on DVE/gpsimd),
    which drops the 1.3us ACT_TABLE_LOAD from the critical path.
  * Group sizes decrease toward the end ([256,512,512,1024,1024,256,256,
    256]) so the drain after the last x byte lands is short.
  * Far fewer tile buffers than baseline: the end-of-kernel semaphore
    drain/teardown storm scales with live semaphore count (~125ns each).
  * Output is fp16 (host upcasts) halving the out DMA.
"""

import numpy as np

N_CORES = 8
B = 32768
B_LOC = B // N_CORES          # 4096 rows per core
KC = 6                        # full 128-row contraction chunks
KT = 784 - KC * 128           # 16-row tail
H = 100                       # hidden width
O = 10                        # output width
NT = 512                      # PSUM-bank subtile (512 f32 cols per bank)
# batch columns per x DMA: small first group so real matmuls start early
# (right as the warm-up dummies end), big middle groups (fewer sems),
# small tail groups (short drain after the stream ends)
GROUPS = [256, 512, 512, 1024, 1024, 256, 256, 256]
WARMUP_MMS = 9                # ~3.8us of cold-rate PE busy to lift the HAM gate

_COMPILED = {}
LAST_RESULTS = None


def _build_nc():
    import concourse.mybir as mybir
    from concourse import bacc
    from concourse.tile import TileContext

    f32 = mybir.dt.float32
    f16 = mybir.dt.float16

    nc = bacc.Bacc(
        "TRN2", target_bir_lowering=False, debug=False, num_devices=N_CORES
    )
    xt = nc.dram_tensor("xt", [128, KC, B_LOC], f16, kind="ExternalInput")
    w1 = nc.dram_tensor("w1", [128, KC, H], f16, kind="ExternalInput")
    # packed [16, 100 + B_LOC]: W1e tail rows | x tail rows
    wxl = nc.dram_tensor("wxl", [KT, H + B_LOC], f16, kind="ExternalInput")
    # packed [100, 110]: W2 | W3
    w23 = nc.dram_tensor("w23", [H, H + O], f16, kind="ExternalInput")
    # packed [100, 3]: b1 | b2 | b3 (b3 on partitions 0..9)
    bb = nc.dram_tensor("bb", [H, 3], f32, kind="ExternalInput")
    ot = nc.dram_tensor("ot", [O, B_LOC], f16, kind="ExternalOutput")

    add = mybir.AluOpType.add
    amax = mybir.AluOpType.max

    with TileContext(nc) as tc:
        with (
            tc.tile_pool(name="wpool", bufs=1) as wpool,
            tc.tile_pool(name="xpool", bufs=1) as xpool,
            tc.tile_pool(name="epool", bufs=2) as epool,
            tc.tile_pool(name="ppool", bufs=1, space="PSUM") as ppool,
        ):
            # PE warm-up: memset a dummy tile, then dep-free matmuls that
            # keep the PE busy through the HAM activity window while the
            # x stream fills. Results go to a never-read PSUM bank.
            dum = wpool.tile([128, NT], f16, name="dum")
            nc.vector.memset(dum, 0.0)
            pdum = ppool.tile([128, NT], f32, name="pdum")
            for _ in range(WARMUP_MMS):
                nc.tensor.matmul(
                    pdum, lhsT=dum[:, 0:128], rhs=dum, start=True, stop=True
                )

            # weights first on each ring (they gate the first matmuls),
            # then all x group DMAs eagerly round-robin over sync/scalar
            w1_t = wpool.tile([128, KC, H], f16, name="w1t")
            nc.sync.dma_start(out=w1_t, in_=w1.ap())
            wxl_t = wpool.tile([KT, H + B_LOC], f16, name="wxlt")
            nc.scalar.dma_start(out=wxl_t, in_=wxl.ap())
            w23_t = wpool.tile([H, H + O], f16, name="w23t")
            nc.scalar.dma_start(out=w23_t, in_=w23.ap())
            bb_t = wpool.tile([H, 3], f32, name="bbt")
            nc.sync.dma_start(out=bb_t, in_=bb.ap())

            rings = [nc.sync, nc.scalar]
            xg_t = []
            g0 = 0
            for g, ntd in enumerate(GROUPS):
                t = xpool.tile([128, KC, ntd], f16, name=f"xg{g}")
                rings[g % 2].dma_start(out=t, in_=xt.ap()[:, :, g0 : g0 + ntd])
                xg_t.append(t)
                g0 += ntd

            w1l_t = wxl_t[:, 0:H]
            xl_t = wxl_t[:, H : H + B_LOC]
            w2_t = w23_t[:, 0:H]
            w3_t = w23_t[:, H : H + O]
            b1_t = bb_t[:, 0:1]
            b2_t = bb_t[:, 1:2]
            b3_t = bb_t[:O, 2:3]

            g0 = 0
            for g, ntd in enumerate(GROUPS):
                xg = xg_t[g]
                for s0 in range(0, ntd, NT):
                    sw = min(NT, ntd - s0)
                    n0 = g0 + s0
                    ps1 = ppool.tile(
                        [128, NT], f32, tag="ps1", bufs=2, name=f"ps1_{g}_{s0}"
                    )
                    for c in range(KC):
                        nc.tensor.matmul(
                            ps1[:H, :sw],
                            lhsT=w1_t[:, c, :],
                            rhs=xg[:, c, s0 : s0 + sw],
                            start=(c == 0),
                            stop=False,
                        )
                    nc.tensor.matmul(
                        ps1[:H, :sw],
                        lhsT=w1l_t,
                        rhs=xl_t[:, n0 : n0 + sw],
                        start=False,
                        stop=True,
                    )
                    h1 = epool.tile([H, NT], f16, tag="h1", name=f"h1_{g}_{s0}")
                    nc.vector.tensor_scalar(
                        h1[:, :sw], ps1[:H, :sw], b1_t, 0.0, add, amax
                    )
                    ps2 = ppool.tile(
                        [128, NT], f32, tag="ps2", bufs=2, name=f"ps2_{g}_{s0}"
                    )
                    nc.tensor.matmul(
                        ps2[:H, :sw], lhsT=w2_t, rhs=h1[:, :sw],
                        start=True, stop=True,
                    )
                    h2 = epool.tile([H, NT], f16, tag="h2", name=f"h2_{g}_{s0}")
                    nc.vector.tensor_scalar(
                        h2[:, :sw], ps2[:H, :sw], b2_t, 0.0, add, amax
                    )
                    ps3 = ppool.tile(
                        [128, NT], f32, tag="ps3", bufs=2, name=f"ps3_{g}_{s0}"
                    )
                    nc.tensor.matmul(
                        ps3[:O, :sw], lhsT=w3_t, rhs=h2[:, :sw],
                        start=True, stop=True,
                    )
                    o_t = epool.tile([O, NT], f16, tag="o_t", name=f"o_{g}_{s0}")
                    nc.vector.tensor_scalar(
                        o_t[:, :sw], ps3[:O, :sw], b3_t, None, add
                    )
                    nc.gpsimd.dma_start(
                        out=ot.ap()[:, n0 : n0 + sw], in_=o_t[:, :sw]
                    )
                g0 += ntd

    nc.finalize()
    return nc


def _fold_conv_into_w1(w_conv, W1):
    """W1e[784, 100] such that x @ W1e == conv3x3(x) @ W1 (exact linear fold)."""
    W1e = np.zeros((28, 28, H), np.float64)
    W1r = W1.astype(np.float64).reshape(26, 26, H)
    wc = w_conv.astype(np.float64)
    for di in range(3):
        for dj in range(3):
            W1e[di : di + 26, dj : dj + 26, :] += wc[di, dj] * W1r
    return W1e.reshape(784, H).astype(np.float32)


def kernel(x, w_conv, W1, b1, W2, b2, W3, b3):
    from concourse.bass_utils import run_bass_kernel_spmd

    global LAST_RESULTS

    x = np.asarray(x, np.float32)
    W1e = _fold_conv_into_w1(np.asarray(w_conv), np.asarray(W1))
    # [784, 100]: rows 0..767 -> [128, KC, 100]; rows 768..783 -> [16, 100]
    w1_dev = np.ascontiguousarray(
        W1e[: KC * 128].reshape(KC, 128, H).transpose(1, 0, 2)
    ).astype(np.float16)
    w1l_dev = W1e[KC * 128 :].astype(np.float16)      # [16, 100]
    w23_dev = np.zeros((H, H + O), np.float16)
    w23_dev[:, 0:H] = np.asarray(W2, np.float32).astype(np.float16)
    w23_dev[:, H : H + O] = np.asarray(W3, np.float32).astype(np.float16)
    bb_dev = np.zeros((H, 3), np.float32)
    bb_dev[:, 0] = np.asarray(b1, np.float32)
    bb_dev[:, 1] = np.asarray(b2, np.float32)
    bb_dev[:O, 2] = np.asarray(b3, np.float32)

    in_maps = []
    for c in range(N_CORES):
        xs = x[c * B_LOC : (c + 1) * B_LOC]          # [B_LOC, 784]
        xT = xs.T.astype(np.float16)                  # [784, B_LOC] fp16
        # main: [128, KC, B_LOC], element [p, k, n] = xT[k*128 + p, n]
        xmain = np.ascontiguousarray(
            xT[: KC * 128].reshape(KC, 128, B_LOC).transpose(1, 0, 2)
        )
        wxl_dev = np.concatenate([w1l_dev, xT[KC * 128 :]], axis=1)
        in_maps.append(
            {
                "xt": xmain,
                "wxl": np.ascontiguousarray(wxl_dev),
                "w1": w1_dev,
                "w23": w23_dev,
                "bb": bb_dev,
            }
        )

    if "nc" not in _COMPILED:
        _COMPILED["nc"] = _build_nc()
    nc = _COMPILED["nc"]

    res = run_bass_kernel_spmd(nc, in_maps, core_ids=list(range(N_CORES)))
    LAST_RESULTS = res

    out = np.empty((B, O), np.float32)
    for c in range(N_CORES):
        out[c * B_LOC : (c + 1) * B_LOC] = res.results[c]["ot"].T.astype(
            np.float32
        )
    return out
